# revision 1
# baseline (speedup 1.0000x reference)
"""CropRandomizer (pos_enc=True) Trainium2 kernel.

Full inputs: images [64,3,240,240] f32, crop_inds_h/w [64,8] i32 (0..23).
Full output: [512, 5, 216, 216] f32 (3 img channels + 2 pos channels, 8
random 216x216 crops per image).

Strategy (data-parallel over 8 NeuronCores, 8 images per core):
- Host prepends the two positional-encoding planes (constant meshgrid) to
  each image -> per-core src [8, 5, 240, 240].
- On device, each plane (image b, channel c) is staged in SBUF twice, split
  into two overlapping 132-row segments: seg0 = rows 0..131 on partition
  24c + 12s + b (s=0), seg1 = rows 108..239 (s=1).  With this layout any
  216-row crop window [h0, h0+216) (h0 <= 24) decomposes into rows
  [h0, h0+108) of seg0 and the same local rows of seg1, so one crop is a
  single 3-dim DMA: partitions [b : b+109 : 12] (10 partitions = (c,s)
  pairs, s fastest), free dims [ds(h0,108), ds(w0,216)].  The destination
  (the output crop) is fully contiguous.
- h0/w0 are loaded from SBUF into sequencer registers at runtime
  (values_load) so one compiled program serves all cores / any offsets.
"""

import numpy as np

import concourse.bacc as bacc
import concourse.bass as bass
import concourse.mybir as mybir
import concourse.tile as tile
from concourse.bass import ds
from concourse.bass_utils import run_bass_kernel_spmd

# Dynamic (register) SBUF AP offsets are lowered as raw linear addresses in
# the 64-bit SBUF map, where consecutive partitions are 256KB (= 65536 f32
# elements) apart — HW-verified by probing.  Static offsets/dim-steps use
# tensor-flat units, so a dynamic AP must carry its partition base in
# hardware units instead.
SBUF_PART_STRIDE_ELEMS = 65536

H = W = 240
CROP = 216
TOP_ROWS = 132          # seg0: rows 0..131
BOT_ROW0 = 108          # seg1: rows 108..239
SEG_ROWS = 108          # rows per crop piece
B_PER_CORE = 8
N_CROPS = 8
CP = 5                  # 3 image channels + 2 pos channels
N_CORES = 8
MAX_OFF = H - CROP - 1  # 23

_PROGRAM = None


def _build_program(repeat=1):
    nc = bacc.Bacc(
        "TRN2", target_bir_lowering=False, debug=False, enable_asserts=False
    )
    src = nc.dram_tensor(
        "src", [B_PER_CORE, CP, H, W], mybir.dt.float32, kind="ExternalInput"
    ).ap()
    ih = nc.dram_tensor(
        "ih", [1, B_PER_CORE * N_CROPS], mybir.dt.int32, kind="ExternalInput"
    ).ap()
    iw = nc.dram_tensor(
        "iw", [1, B_PER_CORE * N_CROPS], mybir.dt.int32, kind="ExternalInput"
    ).ap()
    out = nc.dram_tensor(
        "out",
        [B_PER_CORE * N_CROPS, CP, CROP, CROP],
        mybir.dt.float32,
        kind="ExternalOutput",
    ).ap()

    with tile.TileContext(nc) as tc:
        with tc.tile_pool(name="pool", bufs=1) as pool:
            planes = pool.tile([128, TOP_ROWS, W], mybir.dt.float32)
            ih_t = pool.tile([1, B_PER_CORE * N_CROPS], mybir.dt.int32)
            iw_t = pool.tile([1, B_PER_CORE * N_CROPS], mybir.dt.int32)

            nc.sync.dma_start(ih_t[:], ih[:])
            nc.sync.dma_start(iw_t[:], iw[:])

            # Stage planes: per image b, seg0 partitions {24c+b}, seg1 {24c+12+b}.
            for b in range(B_PER_CORE):
                e0, e1 = (nc.sync, nc.scalar) if b % 2 == 0 else (nc.scalar, nc.sync)
                e0.dma_start(planes[b:b + 97:24, :, :], src[b, :, 0:TOP_ROWS, :])
                e1.dma_start(
                    planes[b + 12:b + 12 + 97:24, :, :], src[b, :, BOT_ROW0:H, :]
                )

            # Crops: iterate n outer / b inner so consecutive in-flight DMAs
            # hit different partition groups (different SBUF ports).
            for j in range(B_PER_CORE * N_CROPS * repeat):
                j = j % (B_PER_CORE * N_CROPS)
                n, b = divmod(j, B_PER_CORE)
                k = b * N_CROPS + n
                eng, dma_eng = (
                    (mybir.EngineType.SP, nc.sync)
                    if j % 2 == 0
                    else (mybir.EngineType.Activation, nc.scalar)
                )
                h0 = nc.values_load(
                    ih_t[0:1, k:k + 1], engines=(eng,),
                    min_val=0, max_val=MAX_OFF, skip_runtime_bounds_check=True,
                )
                w0 = nc.values_load(
                    iw_t[0:1, k:k + 1], engines=(eng,),
                    min_val=0, max_val=MAX_OFF, skip_runtime_bounds_check=True,
                )
                base = planes[0:109:12, ds(h0, SEG_ROWS), ds(w0, CROP)]
                src_ap = bass.AP(
                    tensor=base.tensor,
                    offset=base.offset + b * SBUF_PART_STRIDE_ELEMS,
                    ap=base.ap,
                )
                dma_eng.dma_start(
                    out[k].rearrange("c (s r) w -> (c s) r w", s=2), src_ap
                )

    nc.compile()
    return nc


def _get_program():
    global _PROGRAM
    if _PROGRAM is None:
        _PROGRAM = _build_program()
    return _PROGRAM


def _pos_planes():
    yy, xx = np.meshgrid(
        np.arange(H, dtype=np.float32) / H,
        np.arange(W, dtype=np.float32) / W,
        indexing="ij",
    )
    return np.stack((yy, xx))  # [2, H, W]


def make_in_maps(images, crop_inds_h, crop_inds_w):
    pos = np.broadcast_to(_pos_planes()[None], (B_PER_CORE, 2, H, W))
    in_maps = []
    for c in range(N_CORES):
        sl = slice(c * B_PER_CORE, (c + 1) * B_PER_CORE)
        src = np.ascontiguousarray(
            np.concatenate(
                (np.asarray(images[sl], dtype=np.float32), pos), axis=1
            )
        )
        in_maps.append(
            {
                "src": src,
                "ih": np.ascontiguousarray(
                    np.asarray(crop_inds_h[sl], dtype=np.int32).reshape(1, -1)
                ),
                "iw": np.ascontiguousarray(
                    np.asarray(crop_inds_w[sl], dtype=np.int32).reshape(1, -1)
                ),
            }
        )
    return in_maps


def kernel(images, crop_inds_h, crop_inds_w):
    nc = _get_program()
    in_maps = make_in_maps(images, crop_inds_h, crop_inds_w)
    res = run_bass_kernel_spmd(nc, in_maps, core_ids=list(range(N_CORES)))
    return np.concatenate([r["out"] for r in res.results], axis=0)



# revision 2
# speedup vs baseline: 1.1579x; 1.1579x over previous
"""CropRandomizer (pos_enc=True) Trainium2 kernel — direct DRAM->DRAM crops.

Full inputs: images [64,3,240,240] f32, crop_inds_h/w [64,8] i32 (0..23).
Full output: [512, 5, 216, 216] f32 (3 img channels + 2 pos channels, 8
random 216x216 crops per image).

Strategy (data-parallel over 8 NeuronCores, 8 images per core):
- Host prepends the two positional-encoding planes (constant meshgrid) to
  each image -> per-core src [8, 5, 240, 240] in DRAM, and packs the crop
  offsets interleaved per crop -> idx [1, 128] i32 = [h0,w0, h1,w1, ...].
- Each crop is ONE DRAM->DRAM DMA: src[b, :, h0:h0+216, w0:w0+216] ->
  out[k] (contiguous).  Inner runs are 216 f32 = 864B >= 512B so the DMA
  engines run at the full 360 GB/s bus rate; total transfer time is the
  59.7MB output-write roofline (165.9us).  No SBUF staging of image data
  (the previous version staged 10.1MB/core through SBUF, serializing an
  extra ~28us of DMA-engine time; measured 198.2us vs 171.2us now).
- Only the 128 offsets go to SBUF (one 512B DMA).  Each crop's (h0, w0)
  pair is one values_load_multi into sequencer registers; crops alternate
  between the SP and Activation HWDGE queues so the next crop's descriptor
  generation pipelines behind the current crop's data transfer.
- Raw bass (no TileContext): sync is three manual semaphores (idx loaded;
  per-queue crop-DMA completion counts).  Like the tile framework's
  contract, semaphores are assumed zero at NEFF load and are restored to
  zero at the end of every execution (SP drains both queues via wait_ge,
  then clears), so repeated executions stay correct.
"""

import numpy as np

import concourse.bacc as bacc
import concourse.mybir as mybir
from concourse.bass import ds
from concourse.bass_utils import run_bass_kernel_spmd

H = W = 240
CROP = 216
B_PER_CORE = 8
N_CROPS = 8
CP = 5                  # 3 image channels + 2 pos channels
N_CORES = 8
MAX_OFF = H - CROP - 1  # 23
NIDX = B_PER_CORE * N_CROPS  # 64 crops per core

_PROGRAM = None


def _build_program():
    nc = bacc.Bacc(
        "TRN2", target_bir_lowering=False, debug=False, enable_asserts=False
    )
    src = nc.dram_tensor(
        "src", [B_PER_CORE, CP, H, W], mybir.dt.float32, kind="ExternalInput"
    ).ap()
    idx = nc.dram_tensor(
        "idx", [1, 2 * NIDX], mybir.dt.int32, kind="ExternalInput"
    ).ap()
    out = nc.dram_tensor(
        "out",
        [NIDX, CP, CROP, CROP],
        mybir.dt.float32,
        kind="ExternalOutput",
    ).ap()

    with nc.sbuf_tensor([1, 2 * NIDX], mybir.dt.int32) as idx_th:
        idx_t = idx_th.ap()
        sem_idx = nc.alloc_semaphore("sem_idx")
        sem_s = nc.alloc_semaphore("sem_s")
        sem_a = nc.alloc_semaphore("sem_a")

        nc.sync.dma_start(idx_t[:], idx[:]).then_inc(sem_idx, 16)
        nc.sync.wait_ge(sem_idx, 16)
        nc.scalar.wait_ge(sem_idx, 16)

        ns = na = 0
        for k in range(NIDX):
            b = k // N_CROPS
            eng, dma_eng = (
                (mybir.EngineType.SP, nc.sync)
                if k % 2 == 0
                else (mybir.EngineType.Activation, nc.scalar)
            )
            _, (h0, w0) = nc.values_load_multi_w_load_instructions(
                idx_t[0:1, 2 * k:2 * k + 2], engines=(eng,),
                min_val=0, max_val=MAX_OFF, skip_runtime_bounds_check=True,
            )
            d = dma_eng.dma_start(out[k], src[b, :, ds(h0, CROP), ds(w0, CROP)])
            if k % 2 == 0:
                ns += 16
                d.then_inc(sem_s, 16)
            else:
                na += 16
                d.then_inc(sem_a, 16)

        # SP drains both queues, then restores every semaphore to zero so
        # the next execution of this program sees the load-time state.
        nc.sync.wait_ge(sem_s, ns)
        nc.sync.wait_ge(sem_a, na)
        nc.sync.sem_clear(sem_idx)
        nc.sync.sem_clear(sem_s)
        nc.sync.sem_clear(sem_a)

    nc.compile()
    return nc


def _get_program():
    global _PROGRAM
    if _PROGRAM is None:
        _PROGRAM = _build_program()
    return _PROGRAM


def _pos_planes():
    yy, xx = np.meshgrid(
        np.arange(H, dtype=np.float32) / H,
        np.arange(W, dtype=np.float32) / W,
        indexing="ij",
    )
    return np.stack((yy, xx))  # [2, H, W]


def make_in_maps(images, crop_inds_h, crop_inds_w):
    pos = np.broadcast_to(_pos_planes()[None], (B_PER_CORE, 2, H, W))
    in_maps = []
    for c in range(N_CORES):
        sl = slice(c * B_PER_CORE, (c + 1) * B_PER_CORE)
        src = np.ascontiguousarray(
            np.concatenate(
                (np.asarray(images[sl], dtype=np.float32), pos), axis=1
            )
        )
        hw = np.stack(
            (
                np.asarray(crop_inds_h[sl], dtype=np.int32).reshape(-1),
                np.asarray(crop_inds_w[sl], dtype=np.int32).reshape(-1),
            ),
            axis=1,
        )  # [64, 2] -> interleaved h0,w0 per crop
        in_maps.append(
            {"src": src, "idx": np.ascontiguousarray(hw.reshape(1, -1))}
        )
    return in_maps


def kernel(images, crop_inds_h, crop_inds_w):
    nc = _get_program()
    in_maps = make_in_maps(images, crop_inds_h, crop_inds_w)
    res = run_bass_kernel_spmd(nc, in_maps, core_ids=list(range(N_CORES)))
    return np.concatenate([r["out"] for r in res.results], axis=0)


# revision 3
# speedup vs baseline: 1.1589x; 1.0009x over previous
"""CropRandomizer (pos_enc=True) Trainium2 kernel — direct DRAM->DRAM crops.

Full inputs: images [64,3,240,240] f32, crop_inds_h/w [64,8] i32 (0..23).
Full output: [512, 5, 216, 216] f32 (3 img channels + 2 pos channels, 8
random 216x216 crops per image).

Strategy (data-parallel over 8 NeuronCores, 8 images per core):
- Host prepends the two positional-encoding planes (constant meshgrid) to
  each image -> per-core src [8, 5, 240, 240] in DRAM, and packs the crop
  offsets interleaved per crop -> idx [1, 128] i32 = [h0,w0, h1,w1, ...].
- Each crop is ONE DRAM->DRAM DMA: src[b, :, h0:h0+216, w0:w0+216] ->
  out[k] (contiguous).  Inner runs are 216 f32 = 864B >= 512B so the DMA
  engines run at the full 360 GB/s bus rate; total transfer time is the
  59.7MB output-write roofline (165.9us).  No SBUF staging of image data
  (the previous version staged 10.1MB/core through SBUF, serializing an
  extra ~28us of DMA-engine time; measured 198.2us vs 171.2us now).
- Only the 128 offsets go to SBUF (one 512B DMA).  Each crop's (h0, w0)
  pair is one values_load_multi into sequencer registers; crops alternate
  between the SP and Activation HWDGE queues so the next crop's descriptor
  generation pipelines behind the current crop's data transfer.
- Raw bass (no TileContext): sync is three manual semaphores (idx loaded;
  per-queue crop-DMA completion counts).  Like the tile framework's
  contract, semaphores are assumed zero at NEFF load and are restored to
  zero at the end of every execution (SP drains both queues via wait_ge,
  then clears), so repeated executions stay correct.
"""

import numpy as np

import concourse.bacc as bacc
import concourse.mybir as mybir
from concourse.bass import ds
from concourse.bass_utils import run_bass_kernel_spmd

H = W = 240
CROP = 216
B_PER_CORE = 8
N_CROPS = 8
CP = 5                  # 3 image channels + 2 pos channels
N_CORES = 8
MAX_OFF = H - CROP - 1  # 23
NIDX = B_PER_CORE * N_CROPS  # 64 crops per core

_PROGRAM = None


def _build_program():
    nc = bacc.Bacc(
        "TRN2", target_bir_lowering=False, debug=False, enable_asserts=False
    )
    src = nc.dram_tensor(
        "src", [B_PER_CORE, CP, H, W], mybir.dt.float32, kind="ExternalInput"
    ).ap()
    idx = nc.dram_tensor(
        "idx", [1, 2 * NIDX], mybir.dt.int32, kind="ExternalInput"
    ).ap()
    out = nc.dram_tensor(
        "out",
        [NIDX, CP, CROP, CROP],
        mybir.dt.float32,
        kind="ExternalOutput",
    ).ap()

    with nc.sbuf_tensor([1, 2 * NIDX], mybir.dt.int32) as idx_th:
        idx_t = idx_th.ap()
        sem_idx = nc.alloc_semaphore("sem_idx")
        sem_s = nc.alloc_semaphore("sem_s")
        sem_a = nc.alloc_semaphore("sem_a")

        nc.sync.dma_start(idx_t[:], idx[:]).then_inc(sem_idx, 16)
        nc.sync.wait_ge(sem_idx, 16)
        nc.scalar.wait_ge(sem_idx, 16)

        ns = na = 0
        for k in range(NIDX):
            b = k // N_CROPS
            eng, dma_eng = (
                (mybir.EngineType.SP, nc.sync)
                if k % 2 == 0
                else (mybir.EngineType.Activation, nc.scalar)
            )
            _, (h0, w0) = nc.values_load_multi_w_load_instructions(
                idx_t[0:1, 2 * k:2 * k + 2], engines=(eng,),
                min_val=0, max_val=MAX_OFF, skip_runtime_bounds_check=True,
            )
            d = dma_eng.dma_start(out[k], src[b, :, ds(h0, CROP), ds(w0, CROP)])
            if k % 2 == 0:
                ns += 16
                d.then_inc(sem_s, 16)
            else:
                na += 16
                d.then_inc(sem_a, 16)

        # Each engine drains its own queue, then restores its semaphores to
        # zero so the next execution of this program sees load-time state.
        # (Act owns the later-finishing queue; doing the waits in parallel
        # keeps the final 900ns DMA-sem propagation off a serial chain.)
        nc.sync.wait_ge(sem_s, ns)
        nc.sync.sem_clear(sem_s)
        nc.sync.sem_clear(sem_idx)
        nc.scalar.wait_ge(sem_a, na)
        nc.scalar.sem_clear(sem_a)

    nc.compile()
    return nc


def _get_program():
    global _PROGRAM
    if _PROGRAM is None:
        _PROGRAM = _build_program()
    return _PROGRAM


def _pos_planes():
    yy, xx = np.meshgrid(
        np.arange(H, dtype=np.float32) / H,
        np.arange(W, dtype=np.float32) / W,
        indexing="ij",
    )
    return np.stack((yy, xx))  # [2, H, W]


def make_in_maps(images, crop_inds_h, crop_inds_w):
    pos = np.broadcast_to(_pos_planes()[None], (B_PER_CORE, 2, H, W))
    in_maps = []
    for c in range(N_CORES):
        sl = slice(c * B_PER_CORE, (c + 1) * B_PER_CORE)
        src = np.ascontiguousarray(
            np.concatenate(
                (np.asarray(images[sl], dtype=np.float32), pos), axis=1
            )
        )
        hw = np.stack(
            (
                np.asarray(crop_inds_h[sl], dtype=np.int32).reshape(-1),
                np.asarray(crop_inds_w[sl], dtype=np.int32).reshape(-1),
            ),
            axis=1,
        )  # [64, 2] -> interleaved h0,w0 per crop
        in_maps.append(
            {"src": src, "idx": np.ascontiguousarray(hw.reshape(1, -1))}
        )
    return in_maps


def kernel(images, crop_inds_h, crop_inds_w):
    nc = _get_program()
    in_maps = make_in_maps(images, crop_inds_h, crop_inds_w)
    res = run_bass_kernel_spmd(nc, in_maps, core_ids=list(range(N_CORES)))
    return np.concatenate([r["out"] for r in res.results], axis=0)


# revision 4
# speedup vs baseline: 1.1735x; 1.0126x over previous
"""CropRandomizer (pos_enc=True) Trainium2 kernel — direct DRAM->DRAM crops.

Full inputs: images [64,3,240,240] f32, crop_inds_h/w [64,8] i32 (0..23).
Full output: [512, 5, 216, 216] f32 (3 img channels + 2 pos channels, 8
random 216x216 crops per image).

Strategy (data-parallel over 8 NeuronCores, 8 images per core):
- Host prepends the two positional-encoding planes (constant meshgrid) to
  each image -> per-core src [8, 5, 240, 240] in DRAM, and packs the crop
  offsets interleaved per crop -> idx [1, 128] i32 = [h0,w0, h1,w1, ...].
- Each crop is ONE DRAM->DRAM DMA: src[b, :, h0:h0+216, w0:w0+216] ->
  out[k] (contiguous).  Inner runs are 216 f32 = 864B >= 512B so the DMA
  engines run at the full 360 GB/s bus rate; total transfer time is the
  59.7MB output-write roofline (165.9us).  No SBUF staging at all: the
  crop offsets are values_load'ed into sequencer registers DIRECTLY from
  the idx DRAM tensor (HW-verified bit-exact incl. boundary offsets), so
  there is no idx DMA, no SBUF tile, and no load semaphore — the first
  crop issues ~2.2us earlier than an SBUF-staged idx would allow.
- Crops alternate between the SP and Activation HWDGE queues so the next
  crop's descriptor generation pipelines behind the current crop's data
  transfer; measured steady state is gap-free at 2592ns per 933KB crop.
- Raw bass (no TileContext); sync is two per-queue completion semaphores.
  Like the tile framework's contract, semaphores are assumed zero at NEFF
  load and each engine drains its own queue then clears its semaphore at
  the end of every execution, so repeated executions stay correct.
"""

import numpy as np

import concourse.bacc as bacc
import concourse.mybir as mybir
from concourse.bass import ds
from concourse.bass_utils import run_bass_kernel_spmd

H = W = 240
CROP = 216
B_PER_CORE = 8
N_CROPS = 8
CP = 5                  # 3 image channels + 2 pos channels
N_CORES = 8
MAX_OFF = H - CROP - 1  # 23
NIDX = B_PER_CORE * N_CROPS  # 64 crops per core

_PROGRAM = None


def _build_program():
    nc = bacc.Bacc(
        "TRN2", target_bir_lowering=False, debug=False, enable_asserts=False
    )
    src = nc.dram_tensor(
        "src", [B_PER_CORE, CP, H, W], mybir.dt.float32, kind="ExternalInput"
    ).ap()
    idx = nc.dram_tensor(
        "idx", [1, 2 * NIDX], mybir.dt.int32, kind="ExternalInput"
    ).ap()
    out = nc.dram_tensor(
        "out",
        [NIDX, CP, CROP, CROP],
        mybir.dt.float32,
        kind="ExternalOutput",
    ).ap()

    sem_s = nc.alloc_semaphore("sem_s")
    sem_a = nc.alloc_semaphore("sem_a")

    ns = na = 0
    for k in range(NIDX):
        b = k // N_CROPS
        eng, dma_eng = (
            (mybir.EngineType.SP, nc.sync)
            if k % 2 == 0
            else (mybir.EngineType.Activation, nc.scalar)
        )
        _, (h0, w0) = nc.values_load_multi_w_load_instructions(
            idx[0:1, 2 * k:2 * k + 2], engines=(eng,),
            min_val=0, max_val=MAX_OFF, skip_runtime_bounds_check=True,
        )
        d = dma_eng.dma_start(out[k], src[b, :, ds(h0, CROP), ds(w0, CROP)])
        if k % 2 == 0:
            ns += 16
            d.then_inc(sem_s, 16)
        else:
            na += 16
            d.then_inc(sem_a, 16)

    # Each engine drains its own queue, then restores its semaphore to
    # zero so the next execution of this program sees load-time state.
    nc.sync.wait_ge(sem_s, ns)
    nc.sync.sem_clear(sem_s)
    nc.scalar.wait_ge(sem_a, na)
    nc.scalar.sem_clear(sem_a)

    nc.compile()
    return nc


def _get_program():
    global _PROGRAM
    if _PROGRAM is None:
        _PROGRAM = _build_program()
    return _PROGRAM


def _pos_planes():
    yy, xx = np.meshgrid(
        np.arange(H, dtype=np.float32) / H,
        np.arange(W, dtype=np.float32) / W,
        indexing="ij",
    )
    return np.stack((yy, xx))  # [2, H, W]


def make_in_maps(images, crop_inds_h, crop_inds_w):
    pos = np.broadcast_to(_pos_planes()[None], (B_PER_CORE, 2, H, W))
    in_maps = []
    for c in range(N_CORES):
        sl = slice(c * B_PER_CORE, (c + 1) * B_PER_CORE)
        src = np.ascontiguousarray(
            np.concatenate(
                (np.asarray(images[sl], dtype=np.float32), pos), axis=1
            )
        )
        hw = np.stack(
            (
                np.asarray(crop_inds_h[sl], dtype=np.int32).reshape(-1),
                np.asarray(crop_inds_w[sl], dtype=np.int32).reshape(-1),
            ),
            axis=1,
        )  # [64, 2] -> interleaved h0,w0 per crop
        in_maps.append(
            {"src": src, "idx": np.ascontiguousarray(hw.reshape(1, -1))}
        )
    return in_maps


def kernel(images, crop_inds_h, crop_inds_w):
    nc = _get_program()
    in_maps = make_in_maps(images, crop_inds_h, crop_inds_w)
    res = run_bass_kernel_spmd(nc, in_maps, core_ids=list(range(N_CORES)))
    return np.concatenate([r["out"] for r in res.results], axis=0)
